# revision 23
# baseline (speedup 1.0000x reference)
"""LocalTrittention TRN2 kernel: 8-core batch-data-parallel Bass/Tile implementation.

Problem (B=64, S=256, HID=4096, H=16, D=256, WINDOW=64):
  q,k1,k2,v1,v2 = hs @ W*.T + b*            (5 projections, per-head split)
  s1 = q @ k1^T ; scores = (s1 @ k2^T) * 1/sqrt(D)   (per (b,h), S==D)
  scores[:, S-WINDOW:] = -inf ; probs = softmax(scores)
  out = probs @ (v1+v2)  -> [B,S,HID]

Sharding: batch (64) split across 8 cores (8 batches/core). Weights replicated.
Host prep: layout only (transpose hs shard and the 5 weight matrices so the
contraction index is partition-major); all FLOPs run on device.

Fused single-phase design (no DRAM intermediates):
  - hs shard SBUF-resident per token-half (1024 tokens = 4 batches,
    double-buffered + prefetched); weights stream per (half, head).
  - All activations/weights in fp16: same effective precision as f32r
    matmuls (11 significant bits, fp32 PSUM accumulation) at half the
    DMA/SBUF cost.  Final rel err ~4e-3 vs the fp32 reference.
  - v1/v2 folded on device: wv tiles = wv1+wv2 (GpSimd add), so only 4
    projection matmul volumes run instead of 5.  Σ_j p_j (v_j + bv) =
    ctx + bv, so the v bias rides through softmax for free.
  - k2/v projected only for the 192 unmasked tokens per batch (-25% of
    those sweeps); v is projected d-major (sharing the projection PSUM
    layout) then PE-transposed to token-major for the ctx matmul.
  - Attention (s1 = q@k1^T, scores = s1@k2^T, softmax over valid 192 cols,
    ctx = probs@v) runs per (batch, head) entirely out of SBUF, emission
    software-pipelined across batches and interleaved with the next head's
    q-sweep so the PE never waits on the DVE/Act softmax chain.
  - Engine placement: PE matmuls; evacuations + exp on Activation (own
    SBUF ports); reductions/reciprocal/ctx-scale/probs-transpose-copy on
    DVE; wv folds + hs loads on GpSimd; weight/out DMA on SP + Activation
    HWDGE queues.  TimelineSim: 3.68 ms span, PE ~96% busy.
"""

import sys, time

sys.path.insert(0, "/opt/trn_rl_repo")

import numpy as np

import concourse.bass as bass
import concourse.tile as tile
from concourse import bacc, mybir
from concourse.masks import make_identity

B, S, HID = 64, 256, 4096
H, D = 16, 256
WINDOW = 64
SV = S - WINDOW  # valid (unmasked) score columns: 192
SCALE = 1.0 / float(np.sqrt(D))

NCORES = 8
BPC = B // NCORES  # batches per core: 8
T = BPC * S  # tokens per core: 2048
KC = HID // 128  # contraction chunks: 32
HT = T // 2  # tokens per half: 1024 (4 batches)
BPH = BPC // 2  # batches per half: 4

F32 = mybir.dt.float32
F32R = mybir.dt.float32r
F16 = mybir.dt.float16
AX = mybir.AxisListType.X
EXP = mybir.ActivationFunctionType.Exp
IDT = mybir.ActivationFunctionType.Identity
CPY = mybir.ActivationFunctionType.Copy


def build_bass(reps=1):
    nc = bacc.Bacc("TRN2", target_bir_lowering=False, debug=True)

    # fp16 activations/weights: same effective precision as f32r matmuls
    # (11 significant bits, fp32 PSUM accumulation) at half the DMA/SBUF cost
    hsT = nc.dram_tensor("hsT", [HID, T], F16, kind="ExternalInput")
    wts = {
        n: nc.dram_tensor(f"w{n}T", [HID, HID], F16, kind="ExternalInput")
        for n in ("q", "k1", "k2", "v1", "v2")
    }
    bqs = {
        n: nc.dram_tensor(f"b{n}", [HID], F32, kind="ExternalInput")
        for n in ("q", "k1", "k2", "v1", "v2")
    }
    outd = nc.dram_tensor("out", [T, HID], F32, kind="ExternalOutput")

    with tile.TileContext(nc) as tc:
        with (
            tc.tile_pool(name="const", bufs=1) as const,
            tc.tile_pool(name="sb", bufs=1) as sb,
            tc.tile_pool(name="wp", bufs=1) as wp,
            tc.tile_pool(name="at", bufs=1) as at,
            tc.tile_pool(name="psp", bufs=1, space="PSUM") as psp,
        ):
            ident = const.tile([128, 128], F32)
            make_identity(nc, ident[:])

            # per-partition bias tiles [128, 32] (o-chunk-major): bias[o] at
            # [o % 128, o // 128]
            bias_pm = {}
            for n in ("q", "k1", "k2"):
                t = const.tile([128, KC], F32, name=f"bias_{n}")
                nc.sync.dma_start(t[:], bqs[n].ap().rearrange("(m p) -> p m", p=128))
                bias_pm[n] = t
            bv1t = const.tile([128, KC], F32)
            nc.sync.dma_start(bv1t[:], bqs["v1"].ap().rearrange("(m p) -> p m", p=128))
            bv2t = const.tile([128, KC], F32)
            nc.sync.dma_start(bv2t[:], bqs["v2"].ap().rearrange("(m p) -> p m", p=128))
            bvsum = const.tile([128, KC], F32)
            nc.vector.tensor_add(bvsum[:], bv1t[:], bv2t[:])

            for _rep in range(reps):

                def load_hs(hf):
                    # hs [128, kc, tg, b2, tok]: tg = 512-token group,
                    # b2 = batch within group, tok = token within batch
                    cols = slice(hf * HT, (hf + 1) * HT)
                    t = sb.tile(
                        [128, KC, 2, 2, 256], F16, tag="hs", bufs=2, name="hs"
                    )
                    for kb in range(8):
                        nc.gpsimd.dma_start(
                            t[:, kb * 4 : (kb + 1) * 4],
                            hsT.ap()[kb * 512 : (kb + 1) * 512, cols].rearrange(
                                "(c p) (g b m) -> p c g b m", p=128, g=2, b=2
                            ),
                        )
                    return t

                hs_next = load_hs(0)
                for hf in range(2):
                    hs = hs_next

                    # per-head state threaded between the emission stages
                    heads_sb = [None] * H  # (qh, k1h, k2h, vh)
                    attn_sb = [None] * H  # (s1r/probs/ptr state per group)

                    def emit_sweep(n, h, dest, bias_t, valid_only=False):
                        """d-major projection sweep of w[n] head h into dest.
                        valid_only: only project tokens < SV per batch (the
                        rest are masked out downstream)."""
                        ocols = slice(h * 256, (h + 1) * 256)
                        pps = [
                            psp.tile([128, 2, 256], F32, tag="pp", bufs=4, name="pp")
                            for _ in range(4)
                        ]
                        tv = SV if valid_only else 256
                        for kb in range(16):
                            wt = wp.tile(
                                [128, 2, 256], F16, tag="wt", bufs=8, name="wt"
                            )
                            nc.sync.dma_start(
                                wt[:],
                                wts[n]
                                .ap()[kb * 256 : (kb + 1) * 256, ocols]
                                .rearrange("(c p) o -> p c o", p=128),
                            )
                            for k in range(2):
                                kc = kb * 2 + k
                                for od in range(2):
                                    for tg in range(2):
                                        nc.tensor.matmul(
                                            pps[od * 2 + tg][:, :, :tv],
                                            wt[:, k, od * 128 : (od + 1) * 128],
                                            hs[:, kc, tg, :, :tv],
                                            start=(kc == 0),
                                            stop=(kc == KC - 1),
                                        )
                        for od in range(2):
                            for tg in range(2):
                                nc.scalar.activation(
                                    dest[:, od, tg, :, :tv],
                                    pps[od * 2 + tg][:, :, :tv],
                                    IDT,
                                    bias=bias_t[:, h * 2 + od : h * 2 + od + 1],
                                    scale=1.0,
                                )

                    def emit_v(h):
                        """v = hs@(wv1+wv2).T d-major (GpSimd fold), then
                        PE-transpose to token-major vh."""
                        ocols = slice(h * 256, (h + 1) * 256)
                        pps = [
                            psp.tile([128, 2, 256], F32, tag="pp", bufs=4, name="vpp")
                            for _ in range(4)
                        ]
                        for kb in range(16):
                            wvp = wp.tile(
                                [128, 2, 2, 256], F16, tag="wvp", bufs=6, name="wvp"
                            )
                            for wi, n in enumerate(("v1", "v2")):
                                nc.scalar.dma_start(
                                    wvp[:, wi],
                                    wts[n]
                                    .ap()[kb * 256 : (kb + 1) * 256, ocols]
                                    .rearrange("(c p) o -> p c o", p=128),
                                )
                            wvt = wp.tile(
                                [128, 2, 256], F16, tag="wvt", bufs=4, name="wvt"
                            )
                            nc.gpsimd.tensor_add(wvt[:], wvp[:, 0], wvp[:, 1])
                            for k in range(2):
                                kc = kb * 2 + k
                                for od in range(2):
                                    for tg in range(2):
                                        nc.tensor.matmul(
                                            pps[od * 2 + tg][:],
                                            wvt[:, k, od * 128 : (od + 1) * 128],
                                            hs[:, kc, tg],
                                            start=(kc == 0),
                                            stop=(kc == KC - 1),
                                        )
                        vTh = at.tile([128, 2, 2, 2, SV], F32, tag="vTh", name="vTh")
                        for od in range(2):
                            for tg in range(2):
                                nc.scalar.activation(
                                    vTh[:, od, tg],
                                    pps[od * 2 + tg][:, :, :SV],
                                    IDT,
                                    bias=bvsum[:, h * 2 + od : h * 2 + od + 1],
                                    scale=1.0,
                                )
                        vh = sb.tile([128, 8, 256], F16, tag="vh", name="vh")
                        for t2 in range(4):
                            pt = psp.tile(
                                [128, 2, 256], F32, tag="pa", bufs=4, name="vt_ps"
                            )
                            for ci in range(2):
                                tcg = t2 * 2 + ci  # global 128-token chunk 0..7
                                tg, b2, c = tcg // 4, (tcg // 2) % 2, tcg % 2
                                # odd chunks hold only the 64 valid tokens
                                # (128:192) in partitions 0:64
                                rows = 128 if c == 0 else SV - 128
                                for od in range(2):
                                    nc.tensor.transpose(
                                        pt[:rows, ci, od * 128 : (od + 1) * 128],
                                        vTh[
                                            :, od, tg, b2,
                                            c * 128 : c * 128 + rows,
                                        ],
                                        ident[:],
                                    )
                            nc.scalar.activation(
                                vh[:, t2 * 2 : t2 * 2 + 2, :], pt[:], CPY
                            )
                        return vh

                    def emit_attn12(h):
                        """s1 + scores + softmax for all 4 batches, pipelined
                        in two groups of two."""
                        qh, k1h, k2h, _ = heads_sb[h]
                        state = []
                        for g in range(2):
                            s1r = at.tile(
                                [128, 2, 2, 256], F16, tag="s1r", name="s1r"
                            )
                            scps = []
                            for pi in range(2):
                                bi = g * 2 + pi
                                tg, b2 = bi // 2, bi % 2
                                s1p = psp.tile(
                                    [128, 2, 256], F32, tag="pa", bufs=4, name="s1p"
                                )
                                for mc in range(2):
                                    for od in range(2):
                                        nc.tensor.matmul(
                                            s1p[:, mc, :],
                                            k1h[
                                                :, od, tg, b2,
                                                mc * 128 : (mc + 1) * 128,
                                            ],
                                            qh[:, od, tg, b2],
                                            start=(od == 0),
                                            stop=(od == 1),
                                        )
                                nc.vector.tensor_scalar_mul(
                                    s1r[:, pi], s1p[:], SCALE
                                )
                            negmax = at.tile(
                                [128, 2, 2], F32, tag="ngm", name="ngm"
                            )
                            sumexp = at.tile(
                                [128, 2, 2], F32, tag="sme", name="sme"
                            )
                            recip = at.tile(
                                [128, 2, 2], F32, tag="rcp", name="rcp"
                            )
                            probs = at.tile(
                                [128, 2, 2, SV], F32, tag="probs", name="probs"
                            )
                            for pi in range(2):
                                bi = g * 2 + pi
                                tg, b2 = bi // 2, bi % 2
                                scp = psp.tile(
                                    [128, 2, 256], F32, tag="pa", bufs=4, name="scp"
                                )
                                for qc in range(2):
                                    for mc in range(2):
                                        nc.tensor.matmul(
                                            scp[:, qc, :],
                                            s1r[:, pi, mc, qc * 128 : (qc + 1) * 128],
                                            k2h[:, mc, tg, b2],
                                            start=(mc == 0),
                                            stop=(mc == 1),
                                        )
                                nc.vector.reduce_max(
                                    negmax[:, pi],
                                    scp[:, :, :SV],
                                    axis=AX,
                                    negate=True,
                                )
                                for qc in range(2):
                                    nc.scalar.activation(
                                        probs[:, pi, qc],
                                        scp[:, qc, :SV],
                                        EXP,
                                        bias=negmax[:, pi, qc : qc + 1],
                                        scale=1.0,
                                        accum_out=sumexp[:, pi, qc : qc + 1],
                                    )
                            nc.vector.reciprocal(recip[:], sumexp[:])
                            state.append((probs, recip))
                        attn_sb[h] = state

                    def emit_attn3(h, hf):
                        """probs transpose + ctx + epilogue for all 4 batches."""
                        vh = heads_sb[h][3]
                        ocols = slice(h * 256, (h + 1) * 256)
                        for g in range(2):
                            probs, recip = attn_sb[h][g]
                            ptr = at.tile(
                                [128, 2, 2, 256], F16, tag="ptr", name="ptr"
                            )
                            ptps = []
                            for pi in range(2):
                                ptp = psp.tile(
                                    [128, 2, 256], F32, tag="pa", bufs=4, name="ptp"
                                )
                                for qc in range(2):
                                    nc.tensor.transpose(
                                        ptp[:, qc, 0:128],
                                        probs[:, pi, qc, 0:128],
                                        ident[:],
                                    )
                                    nc.tensor.transpose(
                                        ptp[:64, qc, 128:256],
                                        probs[:, pi, qc, 128:SV],
                                        ident[:],
                                    )
                                # full-tile copy: [64:128, 128:256] quadrants are
                                # stale ring-slot data, finite, never read by ctx
                                nc.vector.tensor_copy(ptr[:, pi], ptp[:])
                                ptps.append(ptp)
                            for pi in range(2):
                                bi = g * 2 + pi
                                tb = bi * 256
                                cxp = psp.tile(
                                    [128, 2, 256], F32, tag="pa", bufs=4, name="cxp"
                                )
                                for qc in range(2):
                                    nc.tensor.matmul(
                                        cxp[:, qc, :],
                                        ptr[:, pi, qc, 0:128],
                                        vh[:, bi * 2, :],
                                        start=True,
                                        stop=False,
                                    )
                                    nc.tensor.matmul(
                                        cxp[:, qc, :],
                                        ptr[:64, pi, qc, 128:256],
                                        vh[:64, bi * 2 + 1, :],
                                        start=False,
                                        stop=True,
                                    )
                                ctxs = at.tile(
                                    [128, 2, 256], F32, tag="ctxs", bufs=2,
                                    name="ctxs",
                                )
                                for qc in range(2):
                                    nc.vector.tensor_scalar_mul(
                                        ctxs[:, qc, :],
                                        cxp[:, qc, :],
                                        recip[:, pi, qc : qc + 1],
                                    )
                                # on gpsimd: an out-DMA waiting for its ctx
                                # epilogue at the head of the SP queue would
                                # block the next head's weight tiles (PE then
                                # stalls ~5us/head on the k1 ldweights)
                                nc.gpsimd.dma_start(
                                    outd.ap()[
                                        hf * HT + tb : hf * HT + tb + 256, ocols
                                    ].rearrange("(c p) s -> p c s", p=128),
                                    ctxs[:],
                                )

                    for h in range(H + 1):
                        if h < H:
                            qh = sb.tile([128, 2, 2, 2, 256], F16, tag="qh", name="qh")
                            emit_sweep("q", h, qh, bias_pm["q"])
                        if h > 0:
                            emit_attn3(h - 1, hf)
                        if h < H:
                            k1h = sb.tile(
                                [128, 2, 2, 2, 256], F16, tag="k1h", name="k1h"
                            )
                            emit_sweep("k1", h, k1h, bias_pm["k1"])
                            k2h = sb.tile(
                                [128, 2, 2, 2, 256], F16, tag="k2h", name="k2h"
                            )
                            emit_sweep("k2", h, k2h, bias_pm["k2"], valid_only=True)
                            # scores cols >= SV are masked downstream; give the
                            # unprojected tail defined (finite) values
                            nc.gpsimd.memset(k2h[:, :, :, :, SV:], 0.0)
                            vh = emit_v(h)
                            heads_sb[h] = (qh, k1h, k2h, vh)
                            emit_attn12(h)
                            if hf == 0 and h == H - 3:
                                # prefetch the next half's hs behind the
                                # remaining heads' compute (hs is 2-deep)
                                hs_next = load_hs(1)
    nc.compile()
    return nc


# ---------------------------------------------------------------------------
# host-side runner (mirrors bass2jax.run_bass_via_pjrt with device-resident
# inputs; weights replicated across cores rather than concatenated)
# ---------------------------------------------------------------------------

_CACHE = {}


def _run(nc, in_maps, n_cores, replicated=(), time_reps=0):
    import jax
    from jax.sharding import Mesh, PartitionSpec, NamedSharding
    from jax.experimental.shard_map import shard_map
    from concourse.bass2jax import (
        install_neuronx_cc_hook,
        _bass_exec_p,
        partition_id_tensor,
    )

    install_neuronx_cc_hook()

    if nc.dbg_addr is not None:
        assert not nc.dbg_callbacks
        in_maps = [
            {**m, nc.dbg_addr.name: np.zeros((1, 2), np.uint32)} for m in in_maps
        ]

    partition_name = nc.partition_id_tensor.name if nc.partition_id_tensor else None

    in_names, out_names, out_avals, zero_outs = [], [], [], []
    for alloc in nc.m.functions[0].allocations:
        if not isinstance(alloc, mybir.MemoryLocationSet):
            continue
        name = alloc.memorylocations[0].name
        if alloc.kind == "ExternalInput":
            if name != partition_name:
                in_names.append(name)
        elif alloc.kind == "ExternalOutput":
            out_names.append(name)
            shape = tuple(alloc.tensor_shape)
            dtype = mybir.dt.np(alloc.dtype)
            out_avals.append(jax.core.ShapedArray(shape, dtype))
            zero_outs.append(np.zeros(shape, dtype))
    n_params = len(in_names)
    n_outs = len(out_avals)
    param_names = list(in_names)
    in_names = in_names + out_names
    if partition_name is not None:
        in_names.append(partition_name)

    donate = tuple(range(n_params, n_params + n_outs))

    def _body(*args):
        operands = list(args)
        if partition_name is not None:
            operands.append(partition_id_tensor())
        outs = _bass_exec_p.bind(
            *operands,
            out_avals=tuple(out_avals),
            in_names=tuple(in_names),
            out_names=tuple(out_names),
            lowering_input_output_aliases=(),
            sim_require_finite=True,
            sim_require_nnan=True,
            nc=nc,
        )
        return tuple(outs)

    devices = jax.devices()[:n_cores]
    mesh = Mesh(np.asarray(devices), ("core",))
    rep = set(replicated)
    in_specs = tuple(
        PartitionSpec() if nm in rep else PartitionSpec("core")
        for nm in param_names
    ) + (PartitionSpec("core"),) * n_outs
    out_specs = (PartitionSpec("core"),) * len(out_names)
    sharded = jax.jit(
        shard_map(
            _body, mesh=mesh, in_specs=in_specs, out_specs=out_specs, check_rep=False
        ),
        donate_argnums=donate,
        keep_unused=True,
    )

    shard_sh = NamedSharding(mesh, PartitionSpec("core"))
    rep_sh = NamedSharding(mesh, PartitionSpec())
    concat_in = []
    for i, nm in enumerate(param_names):
        if nm in rep:
            concat_in.append(jax.device_put(np.asarray(in_maps[0][nm]), rep_sh))
        else:
            concat_in.append(
                jax.device_put(
                    np.concatenate(
                        [np.asarray(in_maps[c][nm]) for c in range(n_cores)], axis=0
                    ),
                    shard_sh,
                )
            )
    jax.block_until_ready(concat_in)

    def fresh_zeros():
        zs = [
            jax.device_put(np.zeros((n_cores * z.shape[0], *z.shape[1:]), z.dtype), shard_sh)
            for z in zero_outs
        ]
        jax.block_until_ready(zs)
        return zs

    t0 = time.perf_counter()
    out_arrs = jax.block_until_ready(sharded(*concat_in, *fresh_zeros()))
    first_call_s = time.perf_counter() - t0
    results = [
        {
            name: np.asarray(out_arrs[i]).reshape(n_cores, *out_avals[i].shape)[c]
            for i, name in enumerate(out_names)
        }
        for c in range(n_cores)
    ]

    # non-donating variant for timing bursts: zeros stay device-resident and
    # are reused across calls (the kernel writes every output element)
    sharded_nd = jax.jit(
        shard_map(
            _body, mesh=mesh, in_specs=in_specs, out_specs=out_specs, check_rep=False
        ),
        keep_unused=True,
    )
    zs_resident = fresh_zeros()

    def timed_burst(m):
        """Enqueue m executions back-to-back, fetch a few bytes of the last
        one's output. Device serializes the execs, so wall ~= dispatch
        overhead + m * exec_time once m*exec exceeds the RPC window."""
        t0 = time.perf_counter()
        outs = None
        for _ in range(m):
            outs = sharded_nd(*concat_in, *zs_resident)
        for o in outs:
            np.asarray(jax.device_get(o.addressable_shards[0].data[0:1, 0:8]))
        return time.perf_counter() - t0

    times = [timed_burst(1) for _ in range(time_reps)]

    return results, times, first_call_s, timed_burst


def kernel(
    hidden_states,
    wq,
    bq,
    wk1,
    bk1,
    wk2,
    bk2,
    wv1,
    bv1,
    wv2,
    bv2,
    _time_reps=0,
    _reps=1,
):
    hs = np.asarray(hidden_states, dtype=np.float32)
    weights = {
        "q": np.asarray(wq, np.float32),
        "k1": np.asarray(wk1, np.float32),
        "k2": np.asarray(wk2, np.float32),
        "v1": np.asarray(wv1, np.float32),
        "v2": np.asarray(wv2, np.float32),
    }
    biases = {
        "q": np.asarray(bq, np.float32),
        "k1": np.asarray(bk1, np.float32),
        "k2": np.asarray(bk2, np.float32),
        "v1": np.asarray(bv1, np.float32),
        "v2": np.asarray(bv2, np.float32),
    }

    if ("nc", _reps) not in _CACHE:
        _CACHE[("nc", _reps)] = build_bass(_reps)
    nc = _CACHE[("nc", _reps)]

    # host prep: layout (transposes) + fp16 dtype conversion, no arithmetic
    wT = {n: np.ascontiguousarray(w.T).astype(np.float16) for n, w in weights.items()}
    in_maps = []
    for c in range(NCORES):
        shard = hs[c * BPC : (c + 1) * BPC].reshape(T, HID)
        m = {"hsT": np.ascontiguousarray(shard.T).astype(np.float16)}
        for n in ("q", "k1", "k2", "v1", "v2"):
            m[f"w{n}T"] = wT[n]
            m[f"b{n}"] = biases[n]
        in_maps.append(m)

    replicated = [f"w{n}T" for n in weights] + [f"b{n}" for n in biases]
    results, times, first_s, burst = _run(
        nc, in_maps, NCORES, replicated=replicated, time_reps=_time_reps
    )
    kernel._last_times = times
    kernel._first_call_s = first_s
    kernel._burst = burst

    out = np.empty((B, S, HID), np.float32)
    for c in range(NCORES):
        out[c * BPC : (c + 1) * BPC] = results[c]["out"].reshape(BPC, S, HID)
    return out


# revision 26
# speedup vs baseline: 1.0421x; 1.0421x over previous
"""LocalTrittention TRN2 kernel: 8-core batch-data-parallel Bass/Tile implementation.

Problem (B=64, S=256, HID=4096, H=16, D=256, WINDOW=64):
  q,k1,k2,v1,v2 = hs @ W*.T + b*            (5 projections, per-head split)
  s1 = q @ k1^T ; scores = (s1 @ k2^T) * 1/sqrt(D)   (per (b,h), S==D)
  scores[:, S-WINDOW:] = -inf ; probs = softmax(scores)
  out = probs @ (v1+v2)  -> [B,S,HID]

Sharding: batch (64) split across 8 cores (8 batches/core). Weights replicated.
Host prep: layout only (transpose hs shard and the 5 weight matrices so the
contraction index is partition-major); all FLOPs run on device.

Fused single-phase design (no DRAM intermediates):
  - hs shard SBUF-resident per token-half (1024 tokens = 4 batches,
    double-buffered + prefetched); weights stream per (half, head).
  - All activations/weights in fp16: same effective precision as f32r
    matmuls (11 significant bits, fp32 PSUM accumulation) at half the
    DMA/SBUF cost.  Final rel err ~4e-3 vs the fp32 reference.
  - v1/v2 folded on device: wv tiles = wv1+wv2 (GpSimd add), so only 4
    projection matmul volumes run instead of 5.  Σ_j p_j (v_j + bv) =
    ctx + bv, so the v bias rides through softmax for free.
  - k2/v projected only for the 192 unmasked tokens per batch (-25% of
    those sweeps); v is projected d-major (sharing the projection PSUM
    layout) then PE-transposed to token-major for the ctx matmul.
  - Attention (s1 = q@k1^T, scores = s1@k2^T, softmax over valid 192 cols,
    ctx = probs@v) runs per (batch, head) entirely out of SBUF, emission
    software-pipelined across batches and interleaved with the next head's
    q-sweep so the PE never waits on the DVE/Act softmax chain.
  - Engine placement: PE matmuls; evacuations + exp on Activation (own
    SBUF ports); reductions/reciprocal/ctx-scale/probs-transpose-copy on
    DVE; wv folds + hs loads on GpSimd; weight/out DMA on SP + Activation
    HWDGE queues.  TimelineSim: 3.68 ms span, PE ~96% busy.
"""

import sys, time

sys.path.insert(0, "/opt/trn_rl_repo")

import numpy as np

import concourse.bass as bass
import concourse.tile as tile
from concourse import bacc, mybir
from concourse.masks import make_identity

B, S, HID = 64, 256, 4096
H, D = 16, 256
WINDOW = 64
SV = S - WINDOW  # valid (unmasked) score columns: 192
SCALE = 1.0 / float(np.sqrt(D))

NCORES = 8
BPC = B // NCORES  # batches per core: 8
T = BPC * S  # tokens per core: 2048
KC = HID // 128  # contraction chunks: 32
HT = T // 2  # tokens per half: 1024 (4 batches)
BPH = BPC // 2  # batches per half: 4

F32 = mybir.dt.float32
F32R = mybir.dt.float32r
F16 = mybir.dt.float16
AX = mybir.AxisListType.X
EXP = mybir.ActivationFunctionType.Exp
IDT = mybir.ActivationFunctionType.Identity
CPY = mybir.ActivationFunctionType.Copy


def build_bass(reps=1):
    nc = bacc.Bacc("TRN2", target_bir_lowering=False, debug=True)

    # fp16 activations/weights: same effective precision as f32r matmuls
    # (11 significant bits, fp32 PSUM accumulation) at half the DMA/SBUF cost
    hsT = nc.dram_tensor("hsT", [HID, T], F16, kind="ExternalInput")
    wts = {
        n: nc.dram_tensor(f"w{n}T", [HID, HID], F16, kind="ExternalInput")
        for n in ("q", "k1", "k2", "v1", "v2")
    }
    bqs = {
        n: nc.dram_tensor(f"b{n}", [HID], F32, kind="ExternalInput")
        for n in ("q", "k1", "k2", "v1", "v2")
    }
    outd = nc.dram_tensor("out", [T, HID], F32, kind="ExternalOutput")

    with tile.TileContext(nc) as tc:
        with (
            tc.tile_pool(name="const", bufs=1) as const,
            tc.tile_pool(name="sb", bufs=1) as sb,
            tc.tile_pool(name="wp", bufs=1) as wp,
            tc.tile_pool(name="at", bufs=1) as at,
            tc.tile_pool(name="psp", bufs=1, space="PSUM") as psp,
        ):
            ident = const.tile([128, 128], F32)
            make_identity(nc, ident[:])

            # per-partition bias tiles [128, 32] (o-chunk-major): bias[o] at
            # [o % 128, o // 128]
            bias_pm = {}
            for n in ("q", "k1", "k2"):
                t = const.tile([128, KC], F32, name=f"bias_{n}")
                nc.sync.dma_start(t[:], bqs[n].ap().rearrange("(m p) -> p m", p=128))
                bias_pm[n] = t
            bv1t = const.tile([128, KC], F32)
            nc.sync.dma_start(bv1t[:], bqs["v1"].ap().rearrange("(m p) -> p m", p=128))
            bv2t = const.tile([128, KC], F32)
            nc.sync.dma_start(bv2t[:], bqs["v2"].ap().rearrange("(m p) -> p m", p=128))
            bvsum = const.tile([128, KC], F32)
            nc.vector.tensor_add(bvsum[:], bv1t[:], bv2t[:])

            for _rep in range(reps):

                def load_hs(hf):
                    # hs [128, kc, tg, b2, tok]: tg = 512-token group,
                    # b2 = batch within group, tok = token within batch
                    cols = slice(hf * HT, (hf + 1) * HT)
                    t = sb.tile(
                        [128, KC, 2, 2, 256], F16, tag="hs", bufs=2, name="hs"
                    )
                    for kb in range(8):
                        nc.gpsimd.dma_start(
                            t[:, kb * 4 : (kb + 1) * 4],
                            hsT.ap()[kb * 512 : (kb + 1) * 512, cols].rearrange(
                                "(c p) (g b m) -> p c g b m", p=128, g=2, b=2
                            ),
                        )
                    return t

                hs_next = load_hs(0)
                for hf in range(2):
                    hs = hs_next

                    # per-head state threaded between the emission stages
                    heads_sb = [None] * H  # (qh, k1h, k2h, vh)
                    attn_sb = [None] * H  # (s1r/probs/ptr state per group)

                    def emit_sweep(n, h, dest, bias_t, valid_only=False):
                        """d-major projection sweep of w[n] head h into dest.
                        valid_only: only project tokens < SV per batch (the
                        rest are masked out downstream)."""
                        ocols = slice(h * 256, (h + 1) * 256)
                        pps = [
                            psp.tile([128, 2, 256], F32, tag="pp", bufs=4, name="pp")
                            for _ in range(4)
                        ]
                        tv = SV if valid_only else 256
                        for kb in range(16):
                            wt = wp.tile(
                                [128, 2, 256], F16, tag="wt", bufs=8, name="wt"
                            )
                            nc.sync.dma_start(
                                wt[:],
                                wts[n]
                                .ap()[kb * 256 : (kb + 1) * 256, ocols]
                                .rearrange("(c p) o -> p c o", p=128),
                            )
                            for k in range(2):
                                kc = kb * 2 + k
                                for od in range(2):
                                    for tg in range(2):
                                        nc.tensor.matmul(
                                            pps[od * 2 + tg][:, :, :tv],
                                            wt[:, k, od * 128 : (od + 1) * 128],
                                            hs[:, kc, tg, :, :tv],
                                            start=(kc == 0),
                                            stop=(kc == KC - 1),
                                        )
                        for od in range(2):
                            for tg in range(2):
                                nc.scalar.activation(
                                    dest[:, od, tg, :, :tv],
                                    pps[od * 2 + tg][:, :, :tv],
                                    IDT,
                                    bias=bias_t[:, h * 2 + od : h * 2 + od + 1],
                                    scale=1.0,
                                )

                    def emit_v(h):
                        """v = hs@(wv1+wv2).T d-major (GpSimd fold), then
                        PE-transpose to token-major vh."""
                        ocols = slice(h * 256, (h + 1) * 256)
                        pps = [
                            psp.tile([128, 2, 256], F32, tag="pp", bufs=4, name="vpp")
                            for _ in range(4)
                        ]
                        for kb in range(16):
                            wvp = wp.tile(
                                [128, 2, 2, 256], F16, tag="wvp", bufs=6, name="wvp"
                            )
                            for wi, n in enumerate(("v1", "v2")):
                                nc.scalar.dma_start(
                                    wvp[:, wi],
                                    wts[n]
                                    .ap()[kb * 256 : (kb + 1) * 256, ocols]
                                    .rearrange("(c p) o -> p c o", p=128),
                                )
                            wvt = wp.tile(
                                [128, 2, 256], F16, tag="wvt", bufs=4, name="wvt"
                            )
                            nc.gpsimd.tensor_add(wvt[:], wvp[:, 0], wvp[:, 1])
                            for k in range(2):
                                kc = kb * 2 + k
                                for od in range(2):
                                    for tg in range(2):
                                        nc.tensor.matmul(
                                            pps[od * 2 + tg][:],
                                            wvt[:, k, od * 128 : (od + 1) * 128],
                                            hs[:, kc, tg],
                                            start=(kc == 0),
                                            stop=(kc == KC - 1),
                                        )
                        vTh = at.tile([128, 2, 2, 2, SV], F32, tag="vTh", name="vTh")
                        for od in range(2):
                            for tg in range(2):
                                nc.scalar.activation(
                                    vTh[:, od, tg],
                                    pps[od * 2 + tg][:, :, :SV],
                                    IDT,
                                    bias=bvsum[:, h * 2 + od : h * 2 + od + 1],
                                    scale=1.0,
                                )
                        vh = sb.tile([128, 8, 256], F16, tag="vh", name="vh")
                        for t2 in range(4):
                            pt = psp.tile(
                                [128, 2, 256], F32, tag="pa", bufs=4, name="vt_ps"
                            )
                            for ci in range(2):
                                tcg = t2 * 2 + ci  # global 128-token chunk 0..7
                                tg, b2, c = tcg // 4, (tcg // 2) % 2, tcg % 2
                                # odd chunks hold only the 64 valid tokens
                                # (128:192) in partitions 0:64
                                rows = 128 if c == 0 else SV - 128
                                for od in range(2):
                                    nc.tensor.transpose(
                                        pt[:rows, ci, od * 128 : (od + 1) * 128],
                                        vTh[
                                            :, od, tg, b2,
                                            c * 128 : c * 128 + rows,
                                        ],
                                        ident[:],
                                    )
                            nc.scalar.activation(
                                vh[:, t2 * 2 : t2 * 2 + 2, :], pt[:], CPY
                            )
                        return vh

                    def emit_attn12(h):
                        """s1 + scores + softmax for all 4 batches, pipelined
                        in two groups of two."""
                        qh, k1h, k2h, _ = heads_sb[h]
                        state = []
                        for g in range(2):
                            s1r = at.tile(
                                [128, 2, 2, 256], F16, tag="s1r", name="s1r"
                            )
                            scps = []
                            for pi in range(2):
                                bi = g * 2 + pi
                                tg, b2 = bi // 2, bi % 2
                                s1p = psp.tile(
                                    [128, 2, 256], F32, tag="pa", bufs=4, name="s1p"
                                )
                                for mc in range(2):
                                    for od in range(2):
                                        nc.tensor.matmul(
                                            s1p[:, mc, :],
                                            k1h[
                                                :, od, tg, b2,
                                                mc * 128 : (mc + 1) * 128,
                                            ],
                                            qh[:, od, tg, b2],
                                            start=(od == 0),
                                            stop=(od == 1),
                                        )
                                nc.vector.tensor_scalar_mul(
                                    s1r[:, pi], s1p[:], SCALE
                                )
                            negmax = at.tile(
                                [128, 2, 2], F32, tag="ngm", name="ngm"
                            )
                            sumexp = at.tile(
                                [128, 2, 2], F32, tag="sme", name="sme"
                            )
                            recip = at.tile(
                                [128, 2, 2], F32, tag="rcp", name="rcp"
                            )
                            probs = at.tile(
                                [128, 2, 2, SV], F32, tag="probs", name="probs"
                            )
                            for pi in range(2):
                                bi = g * 2 + pi
                                tg, b2 = bi // 2, bi % 2
                                scp = psp.tile(
                                    [128, 2, 256], F32, tag="pa", bufs=4, name="scp"
                                )
                                for qc in range(2):
                                    for mc in range(2):
                                        nc.tensor.matmul(
                                            scp[:, qc, :],
                                            s1r[:, pi, mc, qc * 128 : (qc + 1) * 128],
                                            k2h[:, mc, tg, b2],
                                            start=(mc == 0),
                                            stop=(mc == 1),
                                        )
                                nc.vector.reduce_max(
                                    negmax[:, pi],
                                    scp[:, :, :SV],
                                    axis=AX,
                                    negate=True,
                                )
                                for qc in range(2):
                                    nc.scalar.activation(
                                        probs[:, pi, qc],
                                        scp[:, qc, :SV],
                                        EXP,
                                        bias=negmax[:, pi, qc : qc + 1],
                                        scale=1.0,
                                        accum_out=sumexp[:, pi, qc : qc + 1],
                                    )
                            nc.vector.reciprocal(recip[:], sumexp[:])
                            state.append((probs, recip))
                        attn_sb[h] = state

                    def emit_attn3(h, hf):
                        """probs transpose + ctx + epilogue for all 4 batches."""
                        vh = heads_sb[h][3]
                        ocols = slice(h * 256, (h + 1) * 256)
                        for g in range(2):
                            probs, recip = attn_sb[h][g]
                            ptr = at.tile(
                                [128, 2, 2, 256], F16, tag="ptr", name="ptr"
                            )
                            ptps = []
                            for pi in range(2):
                                ptp = psp.tile(
                                    [128, 2, 256], F32, tag="pa", bufs=4, name="ptp"
                                )
                                for qc in range(2):
                                    nc.tensor.transpose(
                                        ptp[:, qc, 0:128],
                                        probs[:, pi, qc, 0:128],
                                        ident[:],
                                    )
                                    nc.tensor.transpose(
                                        ptp[:64, qc, 128:256],
                                        probs[:, pi, qc, 128:SV],
                                        ident[:],
                                    )
                                # full-tile copy: [64:128, 128:256] quadrants are
                                # stale ring-slot data, finite, never read by ctx
                                nc.vector.tensor_copy(ptr[:, pi], ptp[:])
                                ptps.append(ptp)
                            for pi in range(2):
                                bi = g * 2 + pi
                                tb = bi * 256
                                cxp = psp.tile(
                                    [128, 2, 256], F32, tag="pa", bufs=4, name="cxp"
                                )
                                for qc in range(2):
                                    nc.tensor.matmul(
                                        cxp[:, qc, :],
                                        ptr[:, pi, qc, 0:128],
                                        vh[:, bi * 2, :],
                                        start=True,
                                        stop=False,
                                    )
                                    nc.tensor.matmul(
                                        cxp[:, qc, :],
                                        ptr[:64, pi, qc, 128:256],
                                        vh[:64, bi * 2 + 1, :],
                                        start=False,
                                        stop=True,
                                    )
                                ctxs = at.tile(
                                    [128, 2, 256], F32, tag="ctxs", bufs=2,
                                    name="ctxs",
                                )
                                for qc in range(2):
                                    nc.vector.tensor_scalar_mul(
                                        ctxs[:, qc, :],
                                        cxp[:, qc, :],
                                        recip[:, pi, qc : qc + 1],
                                    )
                                nc.sync.dma_start(
                                    outd.ap()[
                                        hf * HT + tb : hf * HT + tb + 256, ocols
                                    ].rearrange("(c p) s -> p c s", p=128),
                                    ctxs[:],
                                )

                    for h in range(H + 1):
                        if h < H:
                            qh = sb.tile([128, 2, 2, 2, 256], F16, tag="qh", name="qh")
                            emit_sweep("q", h, qh, bias_pm["q"])
                        if h > 0:
                            emit_attn3(h - 1, hf)
                        if h < H:
                            k1h = sb.tile(
                                [128, 2, 2, 2, 256], F16, tag="k1h", name="k1h"
                            )
                            emit_sweep("k1", h, k1h, bias_pm["k1"])
                            k2h = sb.tile(
                                [128, 2, 2, 2, 256], F16, tag="k2h", name="k2h"
                            )
                            emit_sweep("k2", h, k2h, bias_pm["k2"], valid_only=True)
                            # scores cols >= SV are masked downstream; give the
                            # unprojected tail defined (finite) values
                            nc.gpsimd.memset(k2h[:, :, :, :, SV:], 0.0)
                            vh = emit_v(h)
                            heads_sb[h] = (qh, k1h, k2h, vh)
                            emit_attn12(h)
                            if hf == 0 and h == H - 3:
                                # prefetch the next half's hs behind the
                                # remaining heads' compute (hs is 2-deep)
                                hs_next = load_hs(1)
    nc.compile()
    return nc


# ---------------------------------------------------------------------------
# host-side runner (mirrors bass2jax.run_bass_via_pjrt with device-resident
# inputs; weights replicated across cores rather than concatenated)
# ---------------------------------------------------------------------------

_CACHE = {}


def _run(nc, in_maps, n_cores, replicated=(), time_reps=0):
    import jax
    from jax.sharding import Mesh, PartitionSpec, NamedSharding
    from jax.experimental.shard_map import shard_map
    from concourse.bass2jax import (
        install_neuronx_cc_hook,
        _bass_exec_p,
        partition_id_tensor,
    )

    install_neuronx_cc_hook()

    if nc.dbg_addr is not None:
        assert not nc.dbg_callbacks
        in_maps = [
            {**m, nc.dbg_addr.name: np.zeros((1, 2), np.uint32)} for m in in_maps
        ]

    partition_name = nc.partition_id_tensor.name if nc.partition_id_tensor else None

    in_names, out_names, out_avals, zero_outs = [], [], [], []
    for alloc in nc.m.functions[0].allocations:
        if not isinstance(alloc, mybir.MemoryLocationSet):
            continue
        name = alloc.memorylocations[0].name
        if alloc.kind == "ExternalInput":
            if name != partition_name:
                in_names.append(name)
        elif alloc.kind == "ExternalOutput":
            out_names.append(name)
            shape = tuple(alloc.tensor_shape)
            dtype = mybir.dt.np(alloc.dtype)
            out_avals.append(jax.core.ShapedArray(shape, dtype))
            zero_outs.append(np.zeros(shape, dtype))
    n_params = len(in_names)
    n_outs = len(out_avals)
    param_names = list(in_names)
    in_names = in_names + out_names
    if partition_name is not None:
        in_names.append(partition_name)

    donate = tuple(range(n_params, n_params + n_outs))

    def _body(*args):
        operands = list(args)
        if partition_name is not None:
            operands.append(partition_id_tensor())
        outs = _bass_exec_p.bind(
            *operands,
            out_avals=tuple(out_avals),
            in_names=tuple(in_names),
            out_names=tuple(out_names),
            lowering_input_output_aliases=(),
            sim_require_finite=True,
            sim_require_nnan=True,
            nc=nc,
        )
        return tuple(outs)

    devices = jax.devices()[:n_cores]
    mesh = Mesh(np.asarray(devices), ("core",))
    rep = set(replicated)
    in_specs = tuple(
        PartitionSpec() if nm in rep else PartitionSpec("core")
        for nm in param_names
    ) + (PartitionSpec("core"),) * n_outs
    out_specs = (PartitionSpec("core"),) * len(out_names)
    sharded = jax.jit(
        shard_map(
            _body, mesh=mesh, in_specs=in_specs, out_specs=out_specs, check_rep=False
        ),
        donate_argnums=donate,
        keep_unused=True,
    )

    shard_sh = NamedSharding(mesh, PartitionSpec("core"))
    rep_sh = NamedSharding(mesh, PartitionSpec())
    concat_in = []
    for i, nm in enumerate(param_names):
        if nm in rep:
            concat_in.append(jax.device_put(np.asarray(in_maps[0][nm]), rep_sh))
        else:
            concat_in.append(
                jax.device_put(
                    np.concatenate(
                        [np.asarray(in_maps[c][nm]) for c in range(n_cores)], axis=0
                    ),
                    shard_sh,
                )
            )
    jax.block_until_ready(concat_in)

    def fresh_zeros():
        zs = [
            jax.device_put(np.zeros((n_cores * z.shape[0], *z.shape[1:]), z.dtype), shard_sh)
            for z in zero_outs
        ]
        jax.block_until_ready(zs)
        return zs

    t0 = time.perf_counter()
    out_arrs = jax.block_until_ready(sharded(*concat_in, *fresh_zeros()))
    first_call_s = time.perf_counter() - t0
    results = [
        {
            name: np.asarray(out_arrs[i]).reshape(n_cores, *out_avals[i].shape)[c]
            for i, name in enumerate(out_names)
        }
        for c in range(n_cores)
    ]

    # non-donating variant for timing bursts: zeros stay device-resident and
    # are reused across calls (the kernel writes every output element)
    sharded_nd = jax.jit(
        shard_map(
            _body, mesh=mesh, in_specs=in_specs, out_specs=out_specs, check_rep=False
        ),
        keep_unused=True,
    )
    zs_resident = fresh_zeros()

    def timed_burst(m):
        """Enqueue m executions back-to-back, fetch a few bytes of the last
        one's output. Device serializes the execs, so wall ~= dispatch
        overhead + m * exec_time once m*exec exceeds the RPC window."""
        t0 = time.perf_counter()
        outs = None
        for _ in range(m):
            outs = sharded_nd(*concat_in, *zs_resident)
        for o in outs:
            np.asarray(jax.device_get(o.addressable_shards[0].data[0:1, 0:8]))
        return time.perf_counter() - t0

    times = [timed_burst(1) for _ in range(time_reps)]

    return results, times, first_call_s, timed_burst


def kernel(
    hidden_states,
    wq,
    bq,
    wk1,
    bk1,
    wk2,
    bk2,
    wv1,
    bv1,
    wv2,
    bv2,
    _time_reps=0,
    _reps=1,
):
    hs = np.asarray(hidden_states, dtype=np.float32)
    weights = {
        "q": np.asarray(wq, np.float32),
        "k1": np.asarray(wk1, np.float32),
        "k2": np.asarray(wk2, np.float32),
        "v1": np.asarray(wv1, np.float32),
        "v2": np.asarray(wv2, np.float32),
    }
    biases = {
        "q": np.asarray(bq, np.float32),
        "k1": np.asarray(bk1, np.float32),
        "k2": np.asarray(bk2, np.float32),
        "v1": np.asarray(bv1, np.float32),
        "v2": np.asarray(bv2, np.float32),
    }

    if ("nc", _reps) not in _CACHE:
        _CACHE[("nc", _reps)] = build_bass(_reps)
    nc = _CACHE[("nc", _reps)]

    # host prep: layout (transposes) + fp16 dtype conversion, no arithmetic
    wT = {n: np.ascontiguousarray(w.T).astype(np.float16) for n, w in weights.items()}
    in_maps = []
    for c in range(NCORES):
        shard = hs[c * BPC : (c + 1) * BPC].reshape(T, HID)
        m = {"hsT": np.ascontiguousarray(shard.T).astype(np.float16)}
        for n in ("q", "k1", "k2", "v1", "v2"):
            m[f"w{n}T"] = wT[n]
            m[f"b{n}"] = biases[n]
        in_maps.append(m)

    replicated = [f"w{n}T" for n in weights] + [f"b{n}" for n in biases]
    results, times, first_s, burst = _run(
        nc, in_maps, NCORES, replicated=replicated, time_reps=_time_reps
    )
    kernel._last_times = times
    kernel._first_call_s = first_s
    kernel._burst = burst

    out = np.empty((B, S, HID), np.float32)
    for c in range(NCORES):
        out[c * BPC : (c + 1) * BPC] = results[c]["out"].reshape(BPC, S, HID)
    return out
